# revision 10
# baseline (speedup 1.0000x reference)
"""Trainium2 Bass kernel for batched Kabsch alignment (rotation + per-axis scale).

Computes, for each batch b:
    H[b]  = fx[b]^T @ fy[b]                  (3x3 cross-covariance, contraction over C=1024)
    R[b]  = V @ U^T  where H = U S V^T       (via Newton polar iteration on H^T)
    S[b,d] = ||fy[:,d]|| / ||fx[:,d]||

Sharding: batch dim B=16384 split contiguously across 8 NeuronCores (2048 each).
Per-core layout: batch b_local = 16*p + t  (p = SBUF partition 0..127, t = 0..15),
so stats, polar iteration, and outputs all stay partition-resident and the
output DMA writes 576B-contiguous runs per partition.

Stat phase (per 128-batch tile, natural layout [128, 3072] = [p, (c,d)]):
  - 7 H channels on DVE  via fused tensor_tensor_reduce (mult + sum accumulate)
  - 2 H channels on GPSIMD via scalar_tensor_tensor with accum_out
  - 6 norm channels on ACT via activation(Square, accum_out)
Polar phase: determinant-scaled Newton iteration X <- (mu*X + (1/mu)*cof(X)/det)/2
on X0 = H^T, batched over all 2048 matrices as [128, 144] element arrays.
"""

import numpy as np

import concourse.bass as bass
import concourse.bacc as bacc
import concourse.tile as tile
from concourse import mybir
from concourse.bass_utils import run_bass_kernel_spmd

F32 = mybir.dt.float32
Alu = mybir.AluOpType
Act = mybir.ActivationFunctionType

B, C, D = 16384, 1024, 3
NCORES = 8
BCORE = B // NCORES  # 2048
P = 128              # SBUF partitions
T = BCORE // P       # 16 batches per partition
CD = C * D           # 3072

N_ITERS = 10         # determinant-scaled Newton iterations

# H channel (d, e) -> accumulates sum_c fx[c,d] * fy[c,e] = H[d,e].
# Polar array entry (a, b) = X0[a,b] = H[b,a]  => channel (d=b, e=a).
# DVE handles 8 channels; the 9th, (a=2,b=2) <=> H[2,2], is polarized:
# GPSIMD computes fx_2 + fy_2, ACT square-accumulates it into Hq, and the
# polar setup computes H[2,2] = (Hq - sum fx_2^2 - sum fy_2^2) / 2.
DVE_CH = [(0, 0), (1, 0), (2, 0), (0, 1), (1, 1), (2, 1), (0, 2), (1, 2)]  # (a, b)

LAST_RESULT = None


def _sap(ap, off, dims):
    """Custom strided view of an SBUF/DRAM AP keeping its partition dim."""
    return bass.AP(
        tensor=ap.tensor,
        offset=ap.offset + off,
        ap=[list(ap.ap[0])] + [list(d) for d in dims],
    )


def build(nc, reps=1):
    fx = nc.declare_dram_parameter("fx", [BCORE, C, D], F32, isOutput=False)
    fy = nc.declare_dram_parameter("fy", [BCORE, C, D], F32, isOutput=False)
    Ro = nc.declare_dram_parameter("R", [BCORE, D, D], F32, isOutput=True)
    So = nc.declare_dram_parameter("S", [BCORE, D], F32, isOutput=True)

    fxr = fx[:].rearrange("(p t) c d -> p t (c d)", p=P)
    fyr = fy[:].rearrange("(p t) c d -> p t (c d)", p=P)

    with tile.TileContext(nc) as tc:
        with (
            tc.tile_pool(name="ins", bufs=3) as ins,
            tc.tile_pool(name="scr", bufs=2) as scr,
            tc.tile_pool(name="stats", bufs=1) as stats,
            tc.tile_pool(name="pol", bufs=1) as pol,
        ):
          for _rep in range(reps):
            # Stats accumulators, one tile per writing engine.
            Hd = stats.tile([P, T * 8], F32, tag="Hd")  # idx = t*8 + k, k = a + 3*b (b<2), k=6/7 -> (a=0/1, b=2)
            Hq = stats.tile([P, T], F32, tag="Hq")      # polarized channel: sum (fx_2 + fy_2)^2
            Nn = stats.tile([P, T * 6], F32, tag="Nn")  # idx = t*6 + k, k = d (fx), 3+d (fy)

            # ---------------- stat phase ----------------
            for j in range(T):
                fxt = ins.tile([P, CD], F32, tag="fxt")
                fyt = ins.tile([P, CD], F32, tag="fyt")
                nc.sync.dma_start(out=fxt[:], in_=fxr[:, j])
                nc.sync.dma_start(out=fyt[:], in_=fyr[:, j])

                dve_scr = scr.tile([P, C], F32, tag="dve_scr")
                gp_sum = scr.tile([P, C], F32, tag="gp_sum")
                act_scr = scr.tile([P, C], F32, tag="act_scr")

                fxa, fya = fxt[:], fyt[:]
                for k, (a, b) in enumerate(DVE_CH):
                    d, e = b, a
                    idx = j * 8 + k
                    nc.vector.scalar_tensor_tensor(
                        out=dve_scr[:],
                        in0=_sap(fxa, d, [[3, C]]),
                        scalar=0.0,
                        in1=_sap(fya, e, [[3, C]]),
                        op0=Alu.bypass,
                        op1=Alu.mult,
                        accum_out=Hd[:, idx : idx + 1],
                    )
                # polarized channel (d=2, e=2): gp_sum = fx_2 + fy_2
                nc.gpsimd.tensor_add(
                    gp_sum[:], _sap(fxa, 2, [[3, C]]), _sap(fya, 2, [[3, C]])
                )
                nc.scalar.activation(
                    out=act_scr[:],
                    in_=gp_sum[:],
                    func=Act.Square,
                    accum_out=Hq[:, j : j + 1],
                )
                for k in range(6):
                    src = fxa if k < 3 else fya
                    d = k % 3
                    idx = j * 6 + k
                    nc.scalar.activation(
                        out=act_scr[:],
                        in_=_sap(src, d, [[3, C]]),
                        func=Act.Square,
                        accum_out=Nn[:, idx : idx + 1],
                    )

            # ---------------- polar phase ----------------
            # X arrays: [P, 144] with layout (a:48, b:16, t:1).
            Xc = pol.tile([P, 144], F32)
            Xe = pol.tile([P, 400], F32)  # 5x5 cyclic extension (a:80, b:16, t:1)
            T1 = pol.tile([P, 144], F32)
            T2 = pol.tile([P, 144], F32)
            CF = pol.tile([P, 144], F32)
            dt2 = pol.tile([P, 48], F32)
            det = pol.tile([P, T], F32)
            d2 = pol.tile([P, T], F32)
            lg = pol.tile([P, T], F32)
            mu = pol.tile([P, T], F32)
            md = pol.tile([P, T], F32)
            rcp = pol.tile([P, T], F32)
            av = pol.tile([P, T], F32)
            bv = pol.tile([P, T], F32)

            # Xc[a, b, t] <- H[b, a] from stats tiles.
            # b in {0,1}: from Hd (k = a + 3b), dims (a, b, t).
            nc.vector.tensor_copy(
                _sap(Xc[:], 0, [[48, 3], [16, 2], [1, T]]),
                _sap(Hd[:], 0, [[1, 3], [3, 2], [8, T]]),
            )
            # (a in {0,1}, b=2): Hd k=6,7. Xc offsets 32 and 80.
            nc.vector.tensor_copy(
                _sap(Xc[:], 32, [[48, 2], [1, T]]),
                _sap(Hd[:], 6, [[1, 2], [8, T]]),
            )
            # (a=2, b=2): depolarize H[2,2] = (Hq - Nx2 - Ny2) / 2 at Xc offset 128.
            q1 = pol.tile([P, T], F32)
            nc.vector.tensor_sub(q1[:], Hq[:], _sap(Nn[:], 2, [[6, T]]))
            nc.vector.tensor_sub(q1[:], q1[:], _sap(Nn[:], 5, [[6, T]]))
            nc.vector.tensor_scalar_mul(_sap(Xc[:], 128, [[1, T]]), q1[:], 0.5)

            Xc3 = _sap(Xc[:], 0, [[48, 3], [16, 3], [1, T]])
            T13 = _sap(T1[:], 0, [[48, 3], [16, 3], [1, T]])
            T23 = _sap(T2[:], 0, [[48, 3], [16, 3], [1, T]])
            CF3 = _sap(CF[:], 0, [[48, 3], [16, 3], [1, T]])

            for it in range(N_ITERS):
                # Cyclic 5x5 extension of Xc (GPSIMD: 1-input copies, off DVE's back).
                nc.gpsimd.tensor_copy(
                    _sap(Xe[:], 0, [[80, 3], [16, 3], [1, T]]), Xc3
                )
                nc.gpsimd.tensor_copy(
                    _sap(Xe[:], 240, [[80, 2], [16, 3], [1, T]]),
                    _sap(Xc[:], 0, [[48, 2], [16, 3], [1, T]]),
                )
                nc.gpsimd.tensor_copy(
                    _sap(Xe[:], 48, [[80, 5], [16, 2], [1, T]]),
                    _sap(Xe[:], 0, [[80, 5], [16, 2], [1, T]]),
                )
                # Cofactor: cof[a,b] = Xe[a+1,b+1]*Xe[a+2,b+2] - Xe[a+1,b+2]*Xe[a+2,b+1]
                nc.vector.tensor_mul(
                    T13,
                    _sap(Xe[:], 96, [[80, 3], [16, 3], [1, T]]),
                    _sap(Xe[:], 192, [[80, 3], [16, 3], [1, T]]),
                )
                nc.vector.tensor_mul(
                    T23,
                    _sap(Xe[:], 112, [[80, 3], [16, 3], [1, T]]),
                    _sap(Xe[:], 176, [[80, 3], [16, 3], [1, T]]),
                )
                nc.vector.tensor_sub(CF3, T13, T23)
                # det = sum_b X[0,b] * cof[0,b]
                nc.vector.tensor_mul(
                    _sap(dt2[:], 0, [[16, 3], [1, T]]),
                    _sap(Xc[:], 0, [[16, 3], [1, T]]),
                    _sap(CF[:], 0, [[16, 3], [1, T]]),
                )
                nc.vector.tensor_reduce(
                    out=det[:],
                    in_=_sap(dt2[:], 0, [[1, T], [16, 3]]),
                    axis=mybir.AxisListType.X,
                    op=Alu.add,
                )
                # mu = |det|^(-1/3) = exp(-ln(det^2)/6)
                nc.vector.tensor_mul(d2[:], det[:], det[:])
                nc.vector.tensor_scalar_max(d2[:], d2[:], 1e-38)
                nc.scalar.activation(lg[:], d2[:], Act.Ln)
                nc.scalar.activation(mu[:], lg[:], Act.Exp, scale=-1.0 / 6.0)
                # X <- 0.5*mu*X + (0.5/(mu*det)) * cof
                nc.vector.tensor_mul(md[:], mu[:], det[:])
                nc.vector.reciprocal(rcp[:], md[:])
                nc.vector.tensor_scalar_mul(av[:], mu[:], 0.5)
                nc.vector.tensor_scalar_mul(bv[:], rcp[:], 0.5)
                nc.vector.tensor_mul(
                    T13, Xc3, _sap(av[:], 0, [[0, 3], [0, 3], [1, T]])
                )
                nc.vector.tensor_mul(
                    T23, CF3, _sap(bv[:], 0, [[0, 3], [0, 3], [1, T]])
                )
                nc.vector.tensor_add(Xc[:], T1[:], T2[:])

            # ---------------- S = sqrt(sum fy^2 / sum fx^2) ----------------
            rx = pol.tile([P, 48], F32)
            rt = pol.tile([P, 48], F32)
            lgs = pol.tile([P, 48], F32)
            S48 = pol.tile([P, 48], F32)
            nc.vector.reciprocal(
                _sap(rx[:], 0, [[3, T], [1, 3]]), _sap(Nn[:], 0, [[6, T], [1, 3]])
            )
            nc.vector.tensor_mul(
                _sap(rt[:], 0, [[3, T], [1, 3]]),
                _sap(Nn[:], 3, [[6, T], [1, 3]]),
                _sap(rx[:], 0, [[3, T], [1, 3]]),
            )
            nc.scalar.activation(lgs[:], rt[:], Act.Ln)
            nc.scalar.activation(S48[:], lgs[:], Act.Exp, scale=0.5)

            # ---------------- outputs ----------------
            Xout = pol.tile([P, 144], F32)  # layout (t:9, a:3, b:1)
            nc.vector.tensor_copy(
                _sap(Xout[:], 0, [[9, T], [3, 3], [1, 3]]),
                _sap(Xc[:], 0, [[1, T], [48, 3], [16, 3]]),
            )
            nc.sync.dma_start(
                out=Ro[:].rearrange("(p t) a b -> p (t a b)", p=P),
                in_=Xout[:],
            )
            nc.sync.dma_start(
                out=So[:].rearrange("(p t) d -> p (t d)", p=P),
                in_=S48[:],
            )

    return nc


def make_nc(reps=1):
    nc = bacc.Bacc("TRN2", debug=False)
    build(nc, reps=reps)
    nc.finalize()
    return nc


def kernel(fx, fy):
    global LAST_RESULT
    fx = np.ascontiguousarray(np.asarray(fx, dtype=np.float32))
    fy = np.ascontiguousarray(np.asarray(fy, dtype=np.float32))
    assert fx.shape == (B, C, D) and fy.shape == (B, C, D)

    nc = make_nc()
    in_maps = [
        {
            "fx": fx[i * BCORE : (i + 1) * BCORE],
            "fy": fy[i * BCORE : (i + 1) * BCORE],
        }
        for i in range(NCORES)
    ]
    res = run_bass_kernel_spmd(nc, in_maps, list(range(NCORES)))
    LAST_RESULT = res

    def _unshard(name):
        return np.concatenate([np.asarray(res.results[i][name]) for i in range(NCORES)], axis=0)

    return (_unshard("R"), _unshard("S"))
